# revision 29
# baseline (speedup 1.0000x reference)
"""Trainium2 Bass kernel for batched scaled-dot-product attention.

Problem (all fp32):
    q = queries @ Wq + bq          [B=4, N=4096, E=64]   (D_MODEL=768)
    k = keys    @ Wk + bk
    v = values  @ Wv + bv
    out = softmax(q k^T / sqrt(E)) @ v                    [B, N, 64]

Sharding: 8 cores, data-parallel over batch x query-half.  Core c handles
batch b=c//2, query rows [h*2048, (h+1)*2048) with h=c%2; it loads the full
keys/values for its batch (softmax needs every key).

Per-core algorithm (all matmuls fp32r = full-rate reduced-precision fp32):
  1. Transpose inputs 128x128-blockwise on the PE (the contraction dim 768
     must sit on partitions), project with W as the stationary operand.
     This yields qT/kT [64, seq] directly (scores need E on partitions).
     Bias (and the 1/sqrt(E) scale for q) are folded into the PSUM->SBUF
     copy on the ACT engine.
  2. v is projected to vT [64, 4096] and PE-transposed back to natural
     [4096, 64] with a ones column appended (row sums of the unnormalized
     attention weights then fall out of the attn @ v_aug matmul for free).
  3. Attention in S^T layout (keys on partitions - no transpose of the
     4096-wide weight matrix needed): per (k-tile kt, q-group-pair gp),
     S^T [128, 1024] = kT_kt^T qT_gp; P^T = exp(S^T) in one wide ACT op
     (scores ~ N(0,1): exp without max subtraction is safe in fp32);
     oT[g] [65, 512] += va_kt^T P^T half, accumulated over kt in PSUM.
  4. The k/v projection groups are interleaved and attention for query
     groups 0-1 streams inside the prologue (kt becomes available as soon
     as k-group and v-group kt//4 are done); groups 2-3 run right after,
     re-reading kT/qT/va from SBUF.  This keeps PE/ACT/DMA all busy and the
     PE HAM clock warm.
  5. Epilogue: PE-transpose oT to natural [512, 65]; multiply the 64 value
     columns by the reciprocal of the ones-column; DMA out.
"""

import numpy as np

B, N, D, E = 4, 4096, 768, 64
NCORES = 8
HALF = N // 2          # query rows per core
CH = D // 128          # 6 feature chunks of the contraction dim
GT = 4                 # seq tiles per projection group (512-wide moving dim)
GROUP = 128 * GT       # 512
KT = N // 128          # 32 key tiles
QG = HALF // GROUP     # 4 query groups per core
SCALE = 1.0 / 8.0      # 1/sqrt(E)

_CACHE = {}


def _build():
    from contextlib import ExitStack

    import concourse.mybir as mybir
    import concourse.tile as tile
    from concourse import bacc
    from concourse.masks import make_identity

    f32 = mybir.dt.float32
    f32r = mybir.dt.float32r
    EXP = mybir.ActivationFunctionType.Exp
    IDENT = mybir.ActivationFunctionType.Identity

    nc = bacc.Bacc(trn_type="TRN2")
    x_q = nc.dram_tensor("x_q", [D, HALF], f32, kind="ExternalInput")
    x_k = nc.dram_tensor("x_k", [D, N], f32, kind="ExternalInput")
    x_v = nc.dram_tensor("x_v", [D, N], f32, kind="ExternalInput")
    w_q = nc.dram_tensor("w_q", [D, E], f32, kind="ExternalInput")
    w_k = nc.dram_tensor("w_k", [D, E], f32, kind="ExternalInput")
    w_v = nc.dram_tensor("w_v", [D, E], f32, kind="ExternalInput")
    b_q = nc.dram_tensor("b_q", [E], f32, kind="ExternalInput")
    b_k = nc.dram_tensor("b_k", [E], f32, kind="ExternalInput")
    b_v = nc.dram_tensor("b_v", [E], f32, kind="ExternalInput")
    out = nc.dram_tensor("out", [HALF, E], f32, kind="ExternalOutput")

    with tile.TileContext(nc) as tc, ExitStack() as ctx:
        singles = ctx.enter_context(tc.tile_pool(name="singles", bufs=1))
        # q/k weights doubled [W|W] so the projections emit [128, seq] with
        # rows 64-127 duplicating rows 0-63 (gives K=128 full-rate S matmuls;
        # the doubled contraction is folded into a halved q scale).
        wq_sb = singles.tile([128, CH, 2 * E], f32r)
        wk_sb = singles.tile([128, CH, 2 * E], f32r)
        wv_sb = singles.tile([128, CH, E], f32r)
        wstage = singles.tile([128, 3, CH, E], f32)
        for i, w_dr in enumerate((w_q, w_k, w_v)):
            nc.sync.dma_start(
                out=wstage[:, i], in_=w_dr.rearrange("(c p) e -> p c e", p=128))
        for half in range(2):
            nc.vector.tensor_copy(wq_sb[:, :, half * E:(half + 1) * E], wstage[:, 0])
            nc.vector.tensor_copy(wk_sb[:, :, half * E:(half + 1) * E], wstage[:, 1])
        nc.vector.tensor_copy(wv_sb, wstage[:, 2])
        bq2_sb = singles.tile([128, 1], f32)
        bk2_sb = singles.tile([128, 1], f32)
        bv_sb = singles.tile([E, 1], f32)
        for half in range(2):
            nc.sync.dma_start(out=bq2_sb[half * E:(half + 1) * E],
                              in_=b_q.rearrange("(p one) -> p one", one=1))
            nc.sync.dma_start(out=bk2_sb[half * E:(half + 1) * E],
                              in_=b_k.rearrange("(p one) -> p one", one=1))
        nc.sync.dma_start(out=bv_sb, in_=b_v.rearrange("(p one) -> p one", one=1))
        bqs_sb = singles.tile([128, 1], f32)
        nc.scalar.mul(bqs_sb, bq2_sb, SCALE / 2.0)  # bias on the 1/(2*sqrt(E)) scale

        ident = singles.tile([128, 128], f32)
        make_identity(nc, ident)
        ident_r = singles.tile([128, 128], f32r)
        nc.vector.tensor_copy(ident_r, ident)

        qT = singles.tile([128, HALF], f32r)    # q^T / (2 sqrt(E)), doubled rows
        kT = singles.tile([128, N], f32r)       # k^T, doubled rows
        vT = singles.tile([E, N], f32r)
        MA = E + 2                              # av stationary width (even)
        va = singles.tile([128, KT, MA], f32r)  # v natural + two ones columns
        ones_sb = singles.tile([128, 2 * KT], f32)
        nc.vector.memset(ones_sb, 1.0)
        nc.vector.tensor_copy(va[:, :, E:], ones_sb.rearrange("p (k two) -> p k two", two=2))

        pT_pool = ctx.enter_context(tc.tile_pool(name="pT", bufs=3))
        ep_pool = ctx.enter_context(tc.tile_pool(name="epo", bufs=2))
        o_psum = ctx.enter_context(tc.tile_pool(name="o", bufs=1, space="PSUM"))

        def project_group(xn_pool, xT_pool, tp_psum, pj_psum,
                          x_dr, g, w_sb, bias, dst, scale):
            """Project one 512-column group of feature-major x into dst."""
            xT = xT_pool.tile([128, CH, GROUP], f32r, tag="xT")
            nc.sync.dma_start(
                out=xT,
                in_=x_dr[:, g * GROUP:(g + 1) * GROUP].rearrange(
                    "(c p) s -> p c s", p=128).bitcast(f32r),
            )
            mp = w_sb.shape[-1]  # output partitions (128 doubled / 64 for v)
            ps = pj_psum.tile([128, GROUP], f32, tag="pj")
            for c in range(CH):
                nc.tensor.matmul(
                    ps[:mp], lhsT=w_sb[:, c, :], rhs=xT[:, c, :],
                    start=(c == 0), stop=(c == CH - 1))
            nc.scalar.activation(
                dst[:, g * GROUP:(g + 1) * GROUP], ps[:mp], IDENT,
                bias=bias, scale=scale)

        def va_chunk(tp_psum, kt):
            po = tp_psum.tile([128, GT, 128], f32r, tag="tp", name="po")
            nc.tensor.transpose(
                po[:, 0, :E], vT[:, kt * 128:(kt + 1) * 128], ident_r[:E, :E])
            nc.vector.tensor_copy(va[:, kt, 0:E], po[:, 0, :E])

        def attention_step(s_psum, kt, g, oT_g, first, last):
            """S^T + exp + oT accumulate for k-tile kt and query group g."""
            s_ps = s_psum.tile([128, GROUP], f32, tag="s", name="s_ps")
            nc.tensor.matmul(
                s_ps,
                lhsT=kT[:, kt * 128:(kt + 1) * 128],
                rhs=qT[:, g * GROUP:(g + 1) * GROUP],
                start=True, stop=True, skip_group_check=True)
            pT = pT_pool.tile([128, GROUP], f32r, tag="pT")
            nc.scalar.activation(pT, s_ps, EXP)
            nc.tensor.matmul(
                oT_g,
                lhsT=va[:, kt, :],
                rhs=pT,
                start=first, stop=last, skip_group_check=True)

        def epilogue(s_psum, g, oT_g):
            oT_sb = ep_pool.tile([MA, GROUP], f32r, tag="oT_sb")
            nc.scalar.copy(oT_sb, oT_g)
            for j in range(GT):
                op = s_psum.tile([128, GROUP], f32r, tag="s", name="op")
                nc.tensor.transpose(
                    op[:, :MA], oT_sb[:, j * 128:(j + 1) * 128],
                    ident_r[:MA, :MA])
                o_sb = ep_pool.tile([128, MA], f32, tag="o_sb")
                nc.vector.tensor_copy(o_sb, op[:, :MA])
                rec = ep_pool.tile([128, 1], f32, tag="rec")
                nc.vector.reciprocal(rec, o_sb[:, E:E + 1])
                o_fin = ep_pool.tile([128, E], f32, tag="o_fin")
                nc.vector.tensor_scalar_mul(o_fin, o_sb[:, 0:E], rec)
                r0 = g * GROUP + j * 128
                nc.sync.dma_start(out=out[r0:r0 + 128, :], in_=o_fin)

        from contextlib import ExitStack as _ES

        with _ES() as pro:
            xn_pool = pro.enter_context(tc.tile_pool(name="xn", bufs=3))
            xT_pool = pro.enter_context(tc.tile_pool(name="xT", bufs=4))
            tp_psum = pro.enter_context(tc.tile_pool(name="tp", bufs=2, space="PSUM"))
            pj_psum = pro.enter_context(tc.tile_pool(name="pj", bufs=2, space="PSUM"))
            s_a = pro.enter_context(tc.tile_pool(name="sa", bufs=2, space="PSUM"))

            def proj(x_dr, g, w_sb, bias, dst, scale):
                project_group(xn_pool, xT_pool, tp_psum, pj_psum,
                              x_dr, g, w_sb, bias, dst, scale)

            # ---- phase 1: q projection ----
            for g in range(QG):
                proj(x_q, g, wq_sb, bqs_sb, qT, SCALE / 2.0)

            # ---- phase 2: interleaved k/v projections + attention groups 0,1
            oT_a = [o_psum.tile([MA, GROUP], f32, tag=f"oTp{h}", name=f"oTa{h}")
                    for h in range(2)]
            for g in range(N // GROUP):
                proj(x_k, g, wk_sb, bk2_sb, kT, 1.0)
                proj(x_v, g, wv_sb, bv_sb, vT, 1.0)
                for kt in range(GT * g, GT * (g + 1)):
                    va_chunk(tp_psum, kt)
                    for h in range(2):
                        attention_step(s_a, kt, h, oT_a[h],
                                       first=(kt == 0), last=(kt == KT - 1))
            for h in range(2):
                epilogue(s_a, h, oT_a[h])

        # ---- phase 3: attention for groups 2,3 (kT/qT/va all resident) ----
        with _ES() as att:
            s_b = att.enter_context(tc.tile_pool(name="sb", bufs=4, space="PSUM"))
            oT_b = [o_psum.tile([MA, GROUP], f32, tag=f"oTp{h}", name=f"oTb{h}")
                    for h in range(2)]
            for kt in range(KT):
                for h in range(2):
                    attention_step(s_b, kt, 2 + h, oT_b[h],
                                   first=(kt == 0), last=(kt == KT - 1))
            for h in range(2):
                epilogue(s_b, 2 + h, oT_b[h])

    nc.finalize()
    return nc


def get_nc():
    if "nc" not in _CACHE:
        _CACHE["nc"] = _build()
    return _CACHE["nc"]


def make_in_maps(queries, keys, values, Wq, bq, Wk, bk, Wv, bv):
    def f(a):
        return np.ascontiguousarray(np.asarray(a), dtype=np.float32)

    queries, keys, values = f(queries), f(keys), f(values)
    shared = {
        "w_q": f(Wq), "w_k": f(Wk), "w_v": f(Wv),
        "b_q": f(bq), "b_k": f(bk), "b_v": f(bv),
    }
    in_maps = []
    for c in range(NCORES):
        b, h = divmod(c, 2)
        in_maps.append({
            "x_q": np.ascontiguousarray(queries[b, h * HALF:(h + 1) * HALF, :].T),
            "x_k": np.ascontiguousarray(keys[b].T),
            "x_v": np.ascontiguousarray(values[b].T),
            **shared,
        })
    return in_maps


def run(trace=False, **inputs):
    from concourse.bass_utils import run_bass_kernel_spmd

    nc = get_nc()
    in_maps = make_in_maps(**inputs)
    res = run_bass_kernel_spmd(
        nc, in_maps, core_ids=list(range(NCORES)), trace=trace)
    full = np.empty((B, N, E), dtype=np.float32)
    for c in range(NCORES):
        b, h = divmod(c, 2)
        full[b, h * HALF:(h + 1) * HALF, :] = res.results[c]["out"]
    return full, res


def kernel(**inputs):
    full, _ = run(trace=False, **inputs)
    return full


# revision 30
# speedup vs baseline: 1.0606x; 1.0606x over previous
"""Trainium2 Bass kernel for batched scaled-dot-product attention.

Problem (all fp32):
    q = queries @ Wq + bq          [B=4, N=4096, E=64]   (D_MODEL=768)
    k = keys    @ Wk + bk
    v = values  @ Wv + bv
    out = softmax(q k^T / sqrt(E)) @ v                    [B, N, 64]

Sharding: 8 cores, data-parallel over batch x query-half.  Core c handles
batch b=c//2, query rows [h*2048, (h+1)*2048) with h=c%2; it loads the full
keys/values for its batch (softmax needs every key).

Per-core algorithm (all matmuls fp32r = full-rate reduced-precision fp32):
  1. Transpose inputs 128x128-blockwise on the PE (the contraction dim 768
     must sit on partitions), project with W as the stationary operand.
     This yields qT/kT [64, seq] directly (scores need E on partitions).
     Bias (and the 1/sqrt(E) scale for q) are folded into the PSUM->SBUF
     copy on the ACT engine.
  2. v is projected to vT [64, 4096] and PE-transposed back to natural
     [4096, 64] with a ones column appended (row sums of the unnormalized
     attention weights then fall out of the attn @ v_aug matmul for free).
  3. Attention in S^T layout (keys on partitions - no transpose of the
     4096-wide weight matrix needed): per (k-tile kt, q-group-pair gp),
     S^T [128, 1024] = kT_kt^T qT_gp; P^T = exp(S^T) in one wide ACT op
     (scores ~ N(0,1): exp without max subtraction is safe in fp32);
     oT[g] [65, 512] += va_kt^T P^T half, accumulated over kt in PSUM.
  4. The k/v projection groups are interleaved and attention for query
     groups 0-1 streams inside the prologue (kt becomes available as soon
     as k-group and v-group kt//4 are done); groups 2-3 run right after,
     re-reading kT/qT/va from SBUF.  This keeps PE/ACT/DMA all busy and the
     PE HAM clock warm.
  5. Epilogue: PE-transpose oT to natural [512, 65]; multiply the 64 value
     columns by the reciprocal of the ones-column; DMA out.
"""

import numpy as np

B, N, D, E = 4, 4096, 768, 64
NCORES = 8
HALF = N // 2          # query rows per core
CH = D // 128          # 6 feature chunks of the contraction dim
GT = 4                 # seq tiles per projection group (512-wide moving dim)
GROUP = 128 * GT       # 512
KT = N // 128          # 32 key tiles
QG = HALF // GROUP     # 4 query groups per core
SCALE = 1.0 / 8.0      # 1/sqrt(E)

_CACHE = {}


def _build():
    from contextlib import ExitStack

    import concourse.mybir as mybir
    import concourse.tile as tile
    from concourse import bacc
    from concourse.masks import make_identity

    f32 = mybir.dt.float32
    f32r = mybir.dt.float32r
    EXP = mybir.ActivationFunctionType.Exp
    IDENT = mybir.ActivationFunctionType.Identity

    nc = bacc.Bacc(trn_type="TRN2")
    x_q = nc.dram_tensor("x_q", [D, HALF], f32, kind="ExternalInput")
    x_k = nc.dram_tensor("x_k", [D, N], f32, kind="ExternalInput")
    x_v = nc.dram_tensor("x_v", [D, N], f32, kind="ExternalInput")
    w_q = nc.dram_tensor("w_q", [D, E], f32, kind="ExternalInput")
    w_k = nc.dram_tensor("w_k", [D, E], f32, kind="ExternalInput")
    w_v = nc.dram_tensor("w_v", [D, E], f32, kind="ExternalInput")
    b_q = nc.dram_tensor("b_q", [E], f32, kind="ExternalInput")
    b_k = nc.dram_tensor("b_k", [E], f32, kind="ExternalInput")
    b_v = nc.dram_tensor("b_v", [E], f32, kind="ExternalInput")
    out = nc.dram_tensor("out", [HALF, E], f32, kind="ExternalOutput")

    with tile.TileContext(nc) as tc, ExitStack() as ctx:
        singles = ctx.enter_context(tc.tile_pool(name="singles", bufs=1))
        # q/k weights doubled [W|W] so the projections emit [128, seq] with
        # rows 64-127 duplicating rows 0-63 (gives K=128 full-rate S matmuls;
        # the doubled contraction is folded into a halved q scale).
        wq_sb = singles.tile([128, CH, 2 * E], f32r)
        wk_sb = singles.tile([128, CH, 2 * E], f32r)
        wv_sb = singles.tile([128, CH, E], f32r)
        wstage = singles.tile([128, 3, CH, E], f32)
        for i, w_dr in enumerate((w_q, w_k, w_v)):
            nc.sync.dma_start(
                out=wstage[:, i], in_=w_dr.rearrange("(c p) e -> p c e", p=128))
        for half in range(2):
            nc.vector.tensor_copy(wq_sb[:, :, half * E:(half + 1) * E], wstage[:, 0])
            nc.vector.tensor_copy(wk_sb[:, :, half * E:(half + 1) * E], wstage[:, 1])
        nc.vector.tensor_copy(wv_sb, wstage[:, 2])
        bq2_sb = singles.tile([128, 1], f32)
        bk2_sb = singles.tile([128, 1], f32)
        bv_sb = singles.tile([E, 1], f32)
        for half in range(2):
            nc.sync.dma_start(out=bq2_sb[half * E:(half + 1) * E],
                              in_=b_q.rearrange("(p one) -> p one", one=1))
            nc.sync.dma_start(out=bk2_sb[half * E:(half + 1) * E],
                              in_=b_k.rearrange("(p one) -> p one", one=1))
        nc.sync.dma_start(out=bv_sb, in_=b_v.rearrange("(p one) -> p one", one=1))
        bqs_sb = singles.tile([128, 1], f32)
        nc.scalar.mul(bqs_sb, bq2_sb, SCALE / 2.0)  # bias on the 1/(2*sqrt(E)) scale

        ident = singles.tile([128, 128], f32)
        make_identity(nc, ident)
        ident_r = singles.tile([128, 128], f32r)
        nc.vector.tensor_copy(ident_r, ident)

        qT = singles.tile([128, HALF], f32r)    # q^T / (2 sqrt(E)), doubled rows
        kT = singles.tile([128, N], f32r)       # k^T, doubled rows
        vT = singles.tile([E, N], f32r)
        MA = E + 2                              # av stationary width (even)
        va = singles.tile([128, KT, MA], f32r)  # v natural + two ones columns
        ones_sb = singles.tile([128, 2 * KT], f32)
        nc.vector.memset(ones_sb, 1.0)
        nc.vector.tensor_copy(va[:, :, E:], ones_sb.rearrange("p (k two) -> p k two", two=2))

        pT_pool = ctx.enter_context(tc.tile_pool(name="pT", bufs=6))
        ep_pool = ctx.enter_context(tc.tile_pool(name="epo", bufs=2))
        o_psum = ctx.enter_context(tc.tile_pool(name="o", bufs=1, space="PSUM"))

        def project_group(xn_pool, xT_pool, tp_psum, pj_psum,
                          x_dr, g, w_sb, bias, dst, scale):
            """Project one 512-column group of feature-major x into dst."""
            xT = xT_pool.tile([128, CH, GROUP], f32r, tag="xT")
            nc.sync.dma_start(
                out=xT,
                in_=x_dr[:, g * GROUP:(g + 1) * GROUP].rearrange(
                    "(c p) s -> p c s", p=128).bitcast(f32r),
            )
            mp = w_sb.shape[-1]  # output partitions (128 doubled / 64 for v)
            ps = pj_psum.tile([128, GROUP], f32, tag="pj")
            for c in range(CH):
                nc.tensor.matmul(
                    ps[:mp], lhsT=w_sb[:, c, :], rhs=xT[:, c, :],
                    start=(c == 0), stop=(c == CH - 1))
            nc.vector.tensor_scalar(
                dst[:, g * GROUP:(g + 1) * GROUP], ps[:mp], scale, bias,
                mybir.AluOpType.mult, mybir.AluOpType.add)

        def va_chunk(tp_psum, kt):
            po = tp_psum.tile([128, GT, 128], f32r, tag="tp", name="po")
            nc.tensor.transpose(
                po[:, 0, :E], vT[:, kt * 128:(kt + 1) * 128], ident_r[:E, :E])
            nc.vector.tensor_copy(va[:, kt, 0:E], po[:, 0, :E])

        def attention_step(s_psum, kt, g, oT_g, first, last):
            """S^T + exp + oT accumulate for k-tile kt and query group g."""
            s_ps = s_psum.tile([128, GROUP], f32, tag="s", name="s_ps")
            nc.tensor.matmul(
                s_ps,
                lhsT=kT[:, kt * 128:(kt + 1) * 128],
                rhs=qT[:, g * GROUP:(g + 1) * GROUP],
                start=True, stop=True, skip_group_check=True)
            pT = pT_pool.tile([128, GROUP], f32r, tag="pT")
            nc.scalar.activation(pT, s_ps, EXP)
            nc.tensor.matmul(
                oT_g,
                lhsT=va[:, kt, :],
                rhs=pT,
                start=first, stop=last, skip_group_check=True)

        def epilogue(s_psum, g, oT_g):
            oT_sb = ep_pool.tile([MA, GROUP], f32r, tag="oT_sb")
            nc.scalar.copy(oT_sb, oT_g)
            for j in range(GT):
                op = s_psum.tile([128, GROUP], f32r, tag="s", name="op")
                nc.tensor.transpose(
                    op[:, :MA], oT_sb[:, j * 128:(j + 1) * 128],
                    ident_r[:MA, :MA])
                o_sb = ep_pool.tile([128, MA], f32, tag="o_sb")
                nc.vector.tensor_copy(o_sb, op[:, :MA])
                rec = ep_pool.tile([128, 1], f32, tag="rec")
                nc.vector.reciprocal(rec, o_sb[:, E:E + 1])
                o_fin = ep_pool.tile([128, E], f32, tag="o_fin")
                nc.vector.tensor_scalar_mul(o_fin, o_sb[:, 0:E], rec)
                r0 = g * GROUP + j * 128
                nc.sync.dma_start(out=out[r0:r0 + 128, :], in_=o_fin)

        from contextlib import ExitStack as _ES

        with _ES() as pro:
            xn_pool = pro.enter_context(tc.tile_pool(name="xn", bufs=3))
            xT_pool = pro.enter_context(tc.tile_pool(name="xT", bufs=4))
            tp_psum = pro.enter_context(tc.tile_pool(name="tp", bufs=2, space="PSUM"))
            pj_psum = pro.enter_context(tc.tile_pool(name="pj", bufs=2, space="PSUM"))
            s_a = pro.enter_context(tc.tile_pool(name="sa", bufs=2, space="PSUM"))

            def proj(x_dr, g, w_sb, bias, dst, scale):
                project_group(xn_pool, xT_pool, tp_psum, pj_psum,
                              x_dr, g, w_sb, bias, dst, scale)

            # ---- phase 1: q projection ----
            for g in range(QG):
                proj(x_q, g, wq_sb, bqs_sb, qT, SCALE / 2.0)

            # ---- phase 2: interleaved k/v projections + attention groups 0,1
            oT_a = [o_psum.tile([MA, GROUP], f32, tag=f"oTp{h}", name=f"oTa{h}")
                    for h in range(2)]
            for g in range(N // GROUP):
                proj(x_k, g, wk_sb, bk2_sb, kT, 1.0)
                proj(x_v, g, wv_sb, bv_sb, vT, 1.0)
                for kt in range(GT * g, GT * (g + 1)):
                    va_chunk(tp_psum, kt)
                    for h in range(2):
                        attention_step(s_a, kt, h, oT_a[h],
                                       first=(kt == 0), last=(kt == KT - 1))
            for h in range(2):
                epilogue(s_a, h, oT_a[h])

        # ---- phase 3: attention for groups 2,3 (kT/qT/va all resident) ----
        with _ES() as att:
            s_b = att.enter_context(tc.tile_pool(name="sb", bufs=4, space="PSUM"))
            oT_b = [o_psum.tile([MA, GROUP], f32, tag=f"oTp{h}", name=f"oTb{h}")
                    for h in range(2)]
            for kt in range(KT):
                for h in range(2):
                    attention_step(s_b, kt, 2 + h, oT_b[h],
                                   first=(kt == 0), last=(kt == KT - 1))
            for h in range(2):
                epilogue(s_b, 2 + h, oT_b[h])

    nc.finalize()
    return nc


def get_nc():
    if "nc" not in _CACHE:
        _CACHE["nc"] = _build()
    return _CACHE["nc"]


def make_in_maps(queries, keys, values, Wq, bq, Wk, bk, Wv, bv):
    def f(a):
        return np.ascontiguousarray(np.asarray(a), dtype=np.float32)

    queries, keys, values = f(queries), f(keys), f(values)
    shared = {
        "w_q": f(Wq), "w_k": f(Wk), "w_v": f(Wv),
        "b_q": f(bq), "b_k": f(bk), "b_v": f(bv),
    }
    in_maps = []
    for c in range(NCORES):
        b, h = divmod(c, 2)
        in_maps.append({
            "x_q": np.ascontiguousarray(queries[b, h * HALF:(h + 1) * HALF, :].T),
            "x_k": np.ascontiguousarray(keys[b].T),
            "x_v": np.ascontiguousarray(values[b].T),
            **shared,
        })
    return in_maps


def run(trace=False, **inputs):
    from concourse.bass_utils import run_bass_kernel_spmd

    nc = get_nc()
    in_maps = make_in_maps(**inputs)
    res = run_bass_kernel_spmd(
        nc, in_maps, core_ids=list(range(NCORES)), trace=trace)
    full = np.empty((B, N, E), dtype=np.float32)
    for c in range(NCORES):
        b, h = divmod(c, 2)
        full[b, h * HALF:(h + 1) * HALF, :] = res.results[c]["out"]
    return full, res


def kernel(**inputs):
    full, _ = run(trace=False, **inputs)
    return full
